# revision 6
# baseline (speedup 1.0000x reference)
"""CRF loss + Viterbi decode kernel for Trainium2 (8 NeuronCores, Bass/Tile).

Strategy (pure batch data-parallelism, B=256 -> 32 sequences per core):

Device, per core (b = 32 local sequences, T = 32 tags, S = 2048 steps):
  * Viterbi max-recurrence, bit-exact to the f32 reference:
      state m_s[(jq,b), jl] (j = jq*8+jl) lives on 128 partitions.
      Per step: DVE computes score = fl(m + em) [128,8]; PE builds
      cand[(jq,b),(jl,i)] = fl(trans[i,j] + score[b,i]) via one K=128
      matmul writing trans then four K=128 zero-padded selector matmuls
      accumulating score; DVE reduce_max over i gives m_{s+1}.
      The whole m history streams to DRAM; backpointers are re-derived
      bit-exactly on the host (argmax ties -> first index, same as jnp).
  * Denominator (log-partition) via the scaled forward algorithm in
    probability space: alpha_{t+1} = (E^T alpha_t) * exp(em_t) with E =
    exp(trans); one tiny PE matmul + one DVE multiply per step,
    per-batch renormalisation every 16 steps accumulating log-scales.
Host: numerator (gold-path score), backtrace, loss assembly in numpy.

Everything is compiled once per process and cached.
"""

import numpy as np

B, S, T = 256, 2048, 32
NCORES = 8
BL = B // NCORES            # 32 sequences per core
CH = 128                    # steps per SBUF chunk
RENORM = 16                 # denominator renormalisation period

_cache = {}


# ----------------------------------------------------------------------------
# numpy fallback (also the host-side reference for off-spec inputs)
# ----------------------------------------------------------------------------
def _np_reference(pred, mask, labels, start, end, trans):
    pred = np.asarray(pred, np.float32)
    mask = np.asarray(mask)
    labels = np.asarray(labels)
    start = np.asarray(start, np.float32)
    end = np.asarray(end, np.float32)
    trans = np.asarray(trans, np.float32)
    b, s, t = pred.shape
    bidx = np.arange(b)

    # viterbi
    score = start[None, :] + pred[:, 0]
    ident = np.broadcast_to(np.arange(t, dtype=np.int32)[None], (b, t))
    history = []
    for st in range(1, s):
        cand = score[:, :, None] + trans[None]          # [b, i, j]
        best_prev = cand.argmax(axis=1).astype(np.int32)
        nxt = cand.max(axis=1) + pred[:, st]
        m = mask[:, st].astype(bool)
        score = np.where(m[:, None], nxt, score)
        history.append(np.where(m[:, None], best_prev, ident))
    last_tag = (score + end[None, :]).argmax(axis=1).astype(np.int32)
    tags = np.zeros((b, s), np.int32)
    tags[:, -1] = last_tag
    cur = last_tag
    for st in range(s - 2, -1, -1):
        cur = history[st][bidx, cur]
        tags[:, st] = cur

    # numerator
    sc = start[labels[:, 0]] + pred[bidx, 0, labels[:, 0]]
    trans_sc = trans[labels[:, :-1], labels[:, 1:]]
    em_sc = np.take_along_axis(pred[:, 1:], labels[:, 1:, None], axis=2)[..., 0]
    sc = sc + ((trans_sc + em_sc) * mask[:, 1:].astype(np.float32)).sum(axis=1)
    seq_ends = mask.sum(axis=1).astype(np.int32) - 1
    num = sc + end[labels[bidx, seq_ends]]

    # denominator (f64 logsumexp forward)
    score_d = (start[None, :] + pred[:, 0]).astype(np.float64)
    for st in range(1, s):
        cand = score_d[:, :, None] + trans[None].astype(np.float64) + \
            pred[:, st].astype(np.float64)[:, None, :]
        mx = cand.max(axis=1)
        nxt = mx + np.log(np.exp(cand - mx[:, None, :]).sum(axis=1))
        m = mask[:, st].astype(bool)
        score_d = np.where(m[:, None], nxt, score_d)
    fin = score_d + end[None, :].astype(np.float64)
    mx = fin.max(axis=1)
    den = mx + np.log(np.exp(fin - mx[:, None]).sum(axis=1))

    loss = -np.mean(num.astype(np.float64) - den)
    return tags, np.float32(loss)


# ----------------------------------------------------------------------------
# Tile/walrus workarounds
# ----------------------------------------------------------------------------
def _apply_tile_patches():
    """This walrus build rejects more than one sync-wait on most instructions
    and cannot encode sem waits on the TileContext exit drain at all. Hoist
    excess waits onto same-engine NOPs."""
    from concourse.tile import TileContext
    from concourse.vector_clock import ScopedClock, VectorClock

    def _drain_and_barrier(self, tick_clock, wait_clock):
        gc = tick_clock.global_clock
        n = len(gc)
        for proc in range(n):
            tick = gc[proc]
            if tick > 0:
                vc = VectorClock([0] * n)
                vc.require_at_least(proc, tick)
                nop_inst = self.nc.sync.nop()
                wait_clock.add_sem_waits(nop_inst.ins, ScopedClock({None: vc}))
        self.nc.sync.drain()
        self.nc.all_engine_barrier()
        assert self.sems is not None
        popped = self.nc._tile_sem_poison_stack.pop()
        assert popped is self._sem_poison
        self.nc.clear_and_free_semaphores(list(self.sems.allocated().values()))
        self.nc.all_engine_barrier()

    TileContext._drain_and_barrier = _drain_and_barrier


def _split_sync_waits(nc, max_waits=1):
    from concourse import mybir
    ctr = 0
    for f in nc.m.functions:
        for bb in f.blocks:
            out = []
            for inst in bb.instructions:
                si = inst.sync_info
                waits = list(si.on_wait) if si is not None else []
                if len(waits) > max_waits:
                    excess = waits[:-max_waits]
                    keep = waits[-max_waits:]
                    for i in range(0, len(excess), max_waits):
                        chunk = excess[i:i + max_waits]
                        ctr += 1
                        nop = mybir.InstNoOp(name=f"waitsplit-{ctr}")
                        nop.engine = inst.engine
                        nop.sync_info = mybir.SyncInfo(on_wait=chunk, on_update=[])
                        out.append(nop)
                    inst.sync_info = mybir.SyncInfo(
                        on_wait=keep, on_update=list(si.on_update))
                out.append(inst)
            bb.instructions = out
    return ctr


# ----------------------------------------------------------------------------
# static weight tensors
# ----------------------------------------------------------------------------
def _build_statics(start, trans):
    f32 = np.float32
    # DQ4: four K=128 selector weights, column-stacked [128, 4*128].
    # DQ4[q*32+b, q*128 + jq*32+b] = 1
    dq4 = np.zeros((128, 512), f32)
    for q in range(4):
        for b in range(32):
            for jq in range(4):
                dq4[q * 32 + b, q * 128 + jq * 32 + b] = 1.0
    # TRL [128,128]: rows q' select output quadrant jq
    trl = np.zeros((128, 128), f32)
    for jq in range(4):
        trl[jq, jq * 32:(jq + 1) * 32] = 1.0
    # TR4L [128,256]: row q holds trans[i, q*8+jl] at column jl*32+i
    tr4l = np.zeros((128, 256), f32)
    for q in range(4):
        for jl in range(8):
            tr4l[q, jl * 32:(jl + 1) * 32] = trans[:, q * 8 + jl]
    # ELHS [128,128]: exp(trans) in rows/cols 0..31
    elhs = np.zeros((128, 128), f32)
    elhs[:T, :T] = np.exp(trans.astype(np.float64)).astype(f32)
    idn = np.eye(128, dtype=f32)
    ones = np.ones((128, 128), f32)
    # START128 [(iq,b), il] = start[iq*8+il]
    start128 = np.zeros((128, 8), f32)
    for iq in range(4):
        start128[iq * 32:(iq + 1) * 32, :] = start[iq * 8:(iq + 1) * 8][None, :]
    # ESTART [i, b] = exp(start[i])
    estart = np.repeat(np.exp(start.astype(np.float64)).astype(f32)[:, None], 32, axis=1)
    return dict(DQ4=dq4, TRL=trl, TR4L=tr4l, ELHS=elhs, IDN=idn,
                ONES=ones, START128=start128, ESTART=estart)


# ----------------------------------------------------------------------------
# bass program
# ----------------------------------------------------------------------------
def _build_bass(s_total=S, ch=CH):
    import concourse.bass as bass
    import concourse.tile as tile
    from concourse import mybir
    from contextlib import ExitStack

    f32 = mybir.dt.float32
    nc = bass.Bass()

    pred_d = nc.dram_tensor("pred", [BL, s_total, T], f32, kind="ExternalInput")
    dq4_d = nc.dram_tensor("DQ4", [128, 512], f32, kind="ExternalInput")
    trl_d = nc.dram_tensor("TRL", [128, 128], f32, kind="ExternalInput")
    tr4l_d = nc.dram_tensor("TR4L", [128, 256], f32, kind="ExternalInput")
    elhs_d = nc.dram_tensor("ELHS", [128, 128], f32, kind="ExternalInput")
    idn_d = nc.dram_tensor("IDN", [128, 128], f32, kind="ExternalInput")
    ones_d = nc.dram_tensor("ONES", [128, 128], f32, kind="ExternalInput")
    start128_d = nc.dram_tensor("START128", [128, 8], f32, kind="ExternalInput")
    estart_d = nc.dram_tensor("ESTART", [32, 32], f32, kind="ExternalInput")

    n_renorm = (s_total - 1) // RENORM
    mhist_d = nc.dram_tensor("mhist", [128, s_total * 8], f32, kind="ExternalOutput")
    alpha_d = nc.dram_tensor("alpha_out", [32, 32], f32, kind="ExternalOutput")
    z_d = nc.dram_tensor("z_out", [max(n_renorm, 1), 32], f32, kind="ExternalOutput")

    with tile.TileContext(nc) as tc, ExitStack() as ctx:
        statics = ctx.enter_context(tc.tile_pool(name="statics", bufs=1))
        pred_pool = ctx.enter_context(tc.tile_pool(name="pred", bufs=2))
        em_pool = ctx.enter_context(tc.tile_pool(name="em", bufs=2))
        m_pool = ctx.enter_context(tc.tile_pool(name="m", bufs=2))
        sc_pool = ctx.enter_context(tc.tile_pool(name="sc", bufs=3))
        pem_pool = ctx.enter_context(tc.tile_pool(name="pem", bufs=4))
        small_pool = ctx.enter_context(tc.tile_pool(name="small", bufs=2))
        alpha_pool = ctx.enter_context(tc.tile_pool(name="alpha", bufs=1))
        cand_ps = ctx.enter_context(tc.tile_pool(name="cand", bufs=2, space="PSUM"))
        dps = ctx.enter_context(tc.tile_pool(name="dps", bufs=2, space="PSUM"))
        tps = ctx.enter_context(tc.tile_pool(name="tps", bufs=2, space="PSUM"))
        zps = ctx.enter_context(tc.tile_pool(name="zps", bufs=1, space="PSUM"))

        t_dq4 = statics.tile([128, 512], f32)
        t_trl = statics.tile([128, 128], f32)
        t_tr4l = statics.tile([128, 256], f32)
        t_elhs = statics.tile([128, 128], f32)
        t_idn = statics.tile([128, 128], f32)
        t_ones = statics.tile([128, 128], f32)
        t_start = statics.tile([128, 8], f32)
        t_estart = statics.tile([32, 32], f32)
        for tt, src in [(t_dq4, dq4_d), (t_trl, trl_d), (t_tr4l, tr4l_d),
                        (t_elhs, elhs_d), (t_idn, idn_d), (t_ones, ones_d),
                        (t_start, start128_d), (t_estart, estart_d)]:
            nc.sync.dma_start(tt[:], src[:])

        t_alpha = alpha_pool.tile([128, 32], f32)
        nc.vector.memset(t_alpha[:], 0.0)

        prev_m = None
        prev_em = None
        n_chunks = s_total // ch
        for ci in range(n_chunks):
            c0 = ci * ch
            t_pred = pred_pool.tile([128, ch * T], f32)
            nc.sync.dma_start(
                t_pred[0:BL, :],
                pred_d[:, c0:c0 + ch, :].rearrange("b s t -> b (s t)"))
            t_em = em_pool.tile([128, ch * 8], f32)
            for iq in range(4):
                em_src = bass.AP(tensor=pred_d, offset=c0 * T + iq * 8,
                                 ap=[[s_total * T, BL], [T, ch], [1, 8]])
                nc.sync.dma_start(t_em[iq * 32:(iq + 1) * 32, :], em_src)
            t_m = m_pool.tile([128, ch * 8], f32)
            if ci == 0:
                nc.sync.dma_start(t_m[:, 0:8], start128_d[:])

            for sl in range(ch):
                s = c0 + sl
                sm = s % 4
                # ---- pem transpose + exp, one per 4 steps ----
                if sm == 0:
                    tp = tps.tile([128, 128], f32)
                    nc.tensor.transpose(
                        tp[:], t_pred[:, sl * T: sl * T + 128], t_idn[:])
                    pem = pem_pool.tile([128, 32], f32)
                    nc.scalar.activation(pem[:], tp[:, 0:32],
                                         mybir.ActivationFunctionType.Exp)
                pslice = pem[sm * 32:(sm + 1) * 32, :]

                # ---- denominator ----
                if s == 0:
                    nc.vector.tensor_mul(t_alpha[0:32, :], t_estart[:], pslice)
                else:
                    dp = dps.tile([128, 32], f32)
                    nc.tensor.matmul(dp[:], t_elhs[:], t_alpha[:],
                                     start=True, stop=True)
                    nc.vector.tensor_mul(t_alpha[0:32, :], dp[0:32, :], pslice)
                    if s % RENORM == 0:
                        zb = zps.tile([128, 32], f32)
                        nc.tensor.matmul(zb[:], t_ones[:], t_alpha[:],
                                         start=True, stop=True)
                        t_z = small_pool.tile([1, 32], f32)
                        nc.vector.tensor_copy(t_z[:], zb[0:1, :])
                        k = s // RENORM - 1
                        nc.sync.dma_start(z_d[k:k + 1, :], t_z[:])
                        t_rz = small_pool.tile([32, 32], f32)
                        nc.vector.reciprocal(t_rz[:], zb[0:32, :])
                        nc.vector.tensor_mul(t_alpha[0:32, :],
                                             t_alpha[0:32, :], t_rz[:])

                # ---- viterbi ----
                if s >= 1:
                    if sl >= 1:
                        m_prev = t_m[:, (sl - 1) * 8: sl * 8]
                        em_prev = t_em[:, (sl - 1) * 8: sl * 8]
                    else:
                        m_prev = prev_m[:, (ch - 1) * 8: ch * 8]
                        em_prev = prev_em[:, (ch - 1) * 8: ch * 8]
                    t_sc = sc_pool.tile([128, 8], f32)
                    nc.vector.tensor_add(t_sc[:], m_prev, em_prev)

                    cand = cand_ps.tile([128, 256], f32)
                    nc.tensor.matmul(cand[:], t_trl[:], t_tr4l[:],
                                     start=True, stop=False,
                                     skip_group_check=True)
                    sc_b = bass.AP(tensor=t_sc.tensor, offset=t_sc.offset,
                                   ap=[list(t_sc.ap[0]), [0, 8], [1, 8]])
                    for q in range(4):
                        out_q = bass.AP(tensor=cand.tensor,
                                        offset=cand.offset + q * 8,
                                        ap=[list(cand.ap[0]), [32, 8], [1, 8]])
                        nc.tensor.matmul(out_q, t_dq4[:, q * 128:(q + 1) * 128],
                                         sc_b, start=False, stop=(q == 3),
                                         skip_group_check=True)
                    cand3 = bass.AP(tensor=cand.tensor, offset=cand.offset,
                                    ap=[list(cand.ap[0]), [32, 8], [1, 32]])
                    nc.vector.tensor_reduce(
                        t_m[:, sl * 8:(sl + 1) * 8], cand3,
                        axis=mybir.AxisListType.X, op=mybir.AluOpType.max)

            nc.sync.dma_start(mhist_d[:, c0 * 8:(c0 + ch) * 8], t_m[:])
            prev_m, prev_em = t_m, t_em

        nc.sync.dma_start(alpha_d[:], t_alpha[0:32, :])

    _split_sync_waits(nc)
    return nc


def _get_program(s_total=S, ch=CH):
    key = (s_total, ch)
    if key not in _cache:
        _apply_tile_patches()
        _cache[key] = _build_bass(s_total, ch)
    return _cache[key]


# ----------------------------------------------------------------------------
# host-side finish
# ----------------------------------------------------------------------------
def _host_finish(pred, labels, start, end, trans, m_all, alpha_all, z_all,
                 s_total=S):
    """m_all [NCORES, 128, s_total, 8]; alpha_all [NCORES, 32, 32] (j,b);
    z_all [NCORES, n_renorm, 32]."""
    f32 = np.float32
    bnum = pred.shape[0]
    # scores[s, bglobal, j] = m + em  (bit-exact fl add, same as device)
    m_r = m_all.reshape(NCORES, 4, 32, s_total, 8)          # [c, jq, b, s, jl]
    m_r = m_r.transpose(3, 0, 2, 1, 4).reshape(s_total, bnum, T)  # [s, B, j]
    scores = m_r + pred.transpose(1, 0, 2)                   # f32 add
    # backtrace
    bidx = np.arange(bnum)
    tags = np.zeros((bnum, s_total), np.int32)
    cur = (scores[s_total - 1] + end[None, :]).argmax(axis=1).astype(np.int32)
    tags[:, s_total - 1] = cur
    transT = np.ascontiguousarray(trans)                     # [i, j]
    for s in range(s_total - 2, -1, -1):
        # bp[s+1][b, cur] = argmax_i( scores[s][b, i] + trans[i, cur] )
        cand = scores[s] + transT[:, cur].T                  # [B, i]
        cur = cand.argmax(axis=1).astype(np.int32)
        tags[:, s] = cur

    # numerator (float64 accumulation; loss tolerance is loose)
    sc = start[labels[:, 0]].astype(np.float64) + \
        pred[bidx, 0, labels[:, 0]].astype(np.float64)
    trans_sc = trans[labels[:, :-1], labels[:, 1:]].astype(np.float64)
    em_sc = np.take_along_axis(pred[:, 1:], labels[:, 1:, None],
                               axis=2)[..., 0].astype(np.float64)
    num = sc + (trans_sc + em_sc).sum(axis=1) + \
        end[labels[:, s_total - 1]].astype(np.float64)

    # denominator finish: den[b] = c[b] + log(sum_j alpha[j,b] * exp(end[j]))
    den = np.zeros(bnum, np.float64)
    for c in range(NCORES):
        a = alpha_all[c].astype(np.float64)                  # [j, b]
        w = np.exp(end.astype(np.float64))[:, None]
        logc = np.log(z_all[c].astype(np.float64)).sum(axis=0)
        den[c * BL:(c + 1) * BL] = logc + np.log((a * w).sum(axis=0))

    loss = -np.mean(num - den)
    return tags, f32(loss)


# ----------------------------------------------------------------------------
# entry point
# ----------------------------------------------------------------------------
def kernel(pred, attention_mask, labels, start_transitions, end_transitions,
           transitions):
    pred = np.asarray(pred, np.float32)
    mask = np.asarray(attention_mask)
    labels = np.asarray(labels)
    start = np.asarray(start_transitions, np.float32)
    end = np.asarray(end_transitions, np.float32)
    trans = np.asarray(transitions, np.float32)

    if (pred.shape != (B, S, T) or mask.shape != (B, S)
            or labels.shape != (B, S) or not np.all(mask == 1)):
        return _np_reference(pred, mask, labels, start, end, trans)

    from concourse.bass_utils import run_bass_kernel_spmd

    nc = _get_program()
    statics = _build_statics(start, trans)
    in_maps = []
    for c in range(NCORES):
        im = {"pred": np.ascontiguousarray(pred[c * BL:(c + 1) * BL])}
        im.update(statics)
        in_maps.append(im)
    res = run_bass_kernel_spmd(nc, in_maps, core_ids=list(range(NCORES)))

    m_all = np.stack([r["mhist"].reshape(128, S, 8) for r in res.results])
    alpha_all = np.stack([r["alpha_out"] for r in res.results])
    z_all = np.stack([r["z_out"] for r in res.results])

    tags, loss = _host_finish(pred, labels, start, end, trans,
                              m_all, alpha_all, z_all)
    return tags, loss


# revision 9
# speedup vs baseline: 1.0934x; 1.0934x over previous
"""CRF loss + Viterbi decode kernel for Trainium2 (8 NeuronCores, Bass/Tile).

Strategy (pure batch data-parallelism, B=256 -> 32 sequences per core):

Device, per core (b = 32 local sequences, T = 32 tags, S = 2048 steps):
  * Viterbi max-recurrence, bit-exact to the f32 reference:
      state m_s[(jq,b), jl] (j = jq*8+jl) lives on 128 partitions.
      Per step: DVE computes score = fl(m + em) [128,8]; PE builds
      cand[(jq,b),(jl,i)] = fl(trans[i,j] + score[b,i]) via one K=128
      matmul writing trans then four K=128 zero-padded selector matmuls
      accumulating score; DVE reduce_max over i gives m_{s+1}.
      The whole m history streams to DRAM; backpointers are re-derived
      bit-exactly on the host (argmax ties -> first index, same as jnp).
  * Denominator (log-partition) via the scaled forward algorithm in
    probability space: alpha_{t+1} = (E^T alpha_t) * exp(em_t) with E =
    exp(trans); one tiny PE matmul + one DVE multiply per step,
    per-batch renormalisation every 16 steps accumulating log-scales.
Host: numerator (gold-path score), backtrace, loss assembly in numpy.

Everything is compiled once per process and cached.
"""

import numpy as np

B, S, T = 256, 2048, 32
NCORES = 8
BL = B // NCORES            # 32 sequences per core
CH = 128                    # steps per SBUF chunk
RENORM = 16                 # denominator renormalisation period

_cache = {}


# ----------------------------------------------------------------------------
# numpy fallback (also the host-side reference for off-spec inputs)
# ----------------------------------------------------------------------------
def _np_reference(pred, mask, labels, start, end, trans):
    pred = np.asarray(pred, np.float32)
    mask = np.asarray(mask)
    labels = np.asarray(labels)
    start = np.asarray(start, np.float32)
    end = np.asarray(end, np.float32)
    trans = np.asarray(trans, np.float32)
    b, s, t = pred.shape
    bidx = np.arange(b)

    # viterbi
    score = start[None, :] + pred[:, 0]
    ident = np.broadcast_to(np.arange(t, dtype=np.int32)[None], (b, t))
    history = []
    for st in range(1, s):
        cand = score[:, :, None] + trans[None]          # [b, i, j]
        best_prev = cand.argmax(axis=1).astype(np.int32)
        nxt = cand.max(axis=1) + pred[:, st]
        m = mask[:, st].astype(bool)
        score = np.where(m[:, None], nxt, score)
        history.append(np.where(m[:, None], best_prev, ident))
    last_tag = (score + end[None, :]).argmax(axis=1).astype(np.int32)
    tags = np.zeros((b, s), np.int32)
    tags[:, -1] = last_tag
    cur = last_tag
    for st in range(s - 2, -1, -1):
        cur = history[st][bidx, cur]
        tags[:, st] = cur

    # numerator
    sc = start[labels[:, 0]] + pred[bidx, 0, labels[:, 0]]
    trans_sc = trans[labels[:, :-1], labels[:, 1:]]
    em_sc = np.take_along_axis(pred[:, 1:], labels[:, 1:, None], axis=2)[..., 0]
    sc = sc + ((trans_sc + em_sc) * mask[:, 1:].astype(np.float32)).sum(axis=1)
    seq_ends = mask.sum(axis=1).astype(np.int32) - 1
    num = sc + end[labels[bidx, seq_ends]]

    # denominator (f64 logsumexp forward)
    score_d = (start[None, :] + pred[:, 0]).astype(np.float64)
    for st in range(1, s):
        cand = score_d[:, :, None] + trans[None].astype(np.float64) + \
            pred[:, st].astype(np.float64)[:, None, :]
        mx = cand.max(axis=1)
        nxt = mx + np.log(np.exp(cand - mx[:, None, :]).sum(axis=1))
        m = mask[:, st].astype(bool)
        score_d = np.where(m[:, None], nxt, score_d)
    fin = score_d + end[None, :].astype(np.float64)
    mx = fin.max(axis=1)
    den = mx + np.log(np.exp(fin - mx[:, None]).sum(axis=1))

    loss = -np.mean(num.astype(np.float64) - den)
    return tags, np.float32(loss)


# ----------------------------------------------------------------------------
# Tile/walrus workarounds
# ----------------------------------------------------------------------------
def _apply_tile_patches():
    """This walrus build rejects more than one sync-wait on most instructions
    and cannot encode sem waits on the TileContext exit drain at all. Hoist
    excess waits onto same-engine NOPs."""
    from concourse.tile import TileContext
    from concourse.vector_clock import ScopedClock, VectorClock

    def _drain_and_barrier(self, tick_clock, wait_clock):
        gc = tick_clock.global_clock
        n = len(gc)
        for proc in range(n):
            tick = gc[proc]
            if tick > 0:
                vc = VectorClock([0] * n)
                vc.require_at_least(proc, tick)
                nop_inst = self.nc.sync.nop()
                wait_clock.add_sem_waits(nop_inst.ins, ScopedClock({None: vc}))
        self.nc.sync.drain()
        self.nc.all_engine_barrier()
        assert self.sems is not None
        popped = self.nc._tile_sem_poison_stack.pop()
        assert popped is self._sem_poison
        self.nc.clear_and_free_semaphores(list(self.sems.allocated().values()))
        self.nc.all_engine_barrier()

    TileContext._drain_and_barrier = _drain_and_barrier


def _split_sync_waits(nc, max_waits=1):
    from concourse import mybir
    ctr = 0
    for f in nc.m.functions:
        for bb in f.blocks:
            out = []
            for inst in bb.instructions:
                si = inst.sync_info
                waits = list(si.on_wait) if si is not None else []
                if len(waits) > max_waits:
                    excess = waits[:-max_waits]
                    keep = waits[-max_waits:]
                    for i in range(0, len(excess), max_waits):
                        chunk = excess[i:i + max_waits]
                        ctr += 1
                        nop = mybir.InstNoOp(name=f"waitsplit-{ctr}")
                        nop.engine = inst.engine
                        nop.sync_info = mybir.SyncInfo(on_wait=chunk, on_update=[])
                        out.append(nop)
                    inst.sync_info = mybir.SyncInfo(
                        on_wait=keep, on_update=list(si.on_update))
                out.append(inst)
            bb.instructions = out
    return ctr


# ----------------------------------------------------------------------------
# static weight tensors
# ----------------------------------------------------------------------------
def _build_statics(start, trans):
    f32 = np.float32
    # DQ4: four K=128 selector weights, column-stacked [128, 4*128].
    # DQ4[q*32+b, q*128 + jq*32+b] = 1
    dq4 = np.zeros((128, 512), f32)
    for q in range(4):
        for b in range(32):
            for jq in range(4):
                dq4[q * 32 + b, q * 128 + jq * 32 + b] = 1.0
    # TRP [128,256]: the full trans pattern, trans[i, jq*8+jl] at
    # partition (jq,b), column jl*32+i  (written into PSUM by ScalarE)
    trp = np.zeros((128, 256), f32)
    for jq in range(4):
        for jl in range(8):
            trp[jq * 32:(jq + 1) * 32, jl * 32:(jl + 1) * 32] = \
                trans[:, jq * 8 + jl][None, :]
    # ELHS32 [32,128]: exp(trans) rows i, cols j (cols 32..127 zero)
    elhs = np.zeros((32, 128), f32)
    elhs[:T, :T] = np.exp(trans.astype(np.float64)).astype(f32)
    ones = np.ones((128, 128), f32)
    # START128 [(iq,b), il] = start[iq*8+il]
    start128 = np.zeros((128, 8), f32)
    for iq in range(4):
        start128[iq * 32:(iq + 1) * 32, :] = start[iq * 8:(iq + 1) * 8][None, :]
    # ESTART [i, b] = exp(start[i])
    estart = np.repeat(np.exp(start.astype(np.float64)).astype(f32)[:, None], 32, axis=1)
    return dict(DQ4=dq4, TRP=trp, ELHS=elhs,
                ONES=ones, START128=start128, ESTART=estart)


# ----------------------------------------------------------------------------
# bass program
# ----------------------------------------------------------------------------
def _build_bass(s_total=S, ch=CH):
    import concourse.bass as bass
    import concourse.tile as tile
    from concourse import mybir
    from contextlib import ExitStack

    f32 = mybir.dt.float32
    nc = bass.Bass()

    pred_d = nc.dram_tensor("pred", [BL, s_total, T], f32, kind="ExternalInput")
    dq4_d = nc.dram_tensor("DQ4", [128, 512], f32, kind="ExternalInput")
    trp_d = nc.dram_tensor("TRP", [128, 256], f32, kind="ExternalInput")
    elhs_d = nc.dram_tensor("ELHS", [32, 128], f32, kind="ExternalInput")
    ones_d = nc.dram_tensor("ONES", [128, 128], f32, kind="ExternalInput")
    start128_d = nc.dram_tensor("START128", [128, 8], f32, kind="ExternalInput")
    estart_d = nc.dram_tensor("ESTART", [32, 32], f32, kind="ExternalInput")

    n_renorm = (s_total - 1) // RENORM
    mhist_d = nc.dram_tensor("mhist", [128, s_total * 8], f32, kind="ExternalOutput")
    alpha_d = nc.dram_tensor("alpha_out", [32, 32], f32, kind="ExternalOutput")
    z_d = nc.dram_tensor("z_out", [max(n_renorm, 1), 32], f32, kind="ExternalOutput")

    with tile.TileContext(nc) as tc, ExitStack() as ctx:
        statics = ctx.enter_context(tc.tile_pool(name="statics", bufs=1))
        pred_pool = ctx.enter_context(tc.tile_pool(name="pred", bufs=2))
        em_pool = ctx.enter_context(tc.tile_pool(name="em", bufs=2))
        m_pool = ctx.enter_context(tc.tile_pool(name="m", bufs=2))
        sc_pool = ctx.enter_context(tc.tile_pool(name="sc", bufs=3))
        pem_pool = ctx.enter_context(tc.tile_pool(name="pem", bufs=4))
        small_pool = ctx.enter_context(tc.tile_pool(name="small", bufs=2))
        alpha_pool = ctx.enter_context(tc.tile_pool(name="alpha", bufs=1))
        cand_ps = ctx.enter_context(tc.tile_pool(name="cand", bufs=2, space="PSUM"))
        dps = ctx.enter_context(tc.tile_pool(name="dps", bufs=2, space="PSUM"))
        zps = ctx.enter_context(tc.tile_pool(name="zps", bufs=1, space="PSUM"))

        t_dq4 = statics.tile([128, 512], f32)
        t_trp = statics.tile([128, 256], f32)
        t_elhs = statics.tile([32, 128], f32)
        t_ones = statics.tile([128, 128], f32)
        t_start = statics.tile([128, 8], f32)
        t_estart = statics.tile([32, 32], f32)
        for tt, src in [(t_dq4, dq4_d), (t_trp, trp_d),
                        (t_elhs, elhs_d), (t_ones, ones_d),
                        (t_start, start128_d), (t_estart, estart_d)]:
            nc.sync.dma_start(tt[:], src[:])

        t_alpha = alpha_pool.tile([128, 32], f32)
        nc.vector.memset(t_alpha[:], 0.0)

        prev_m = None
        prev_em = None
        n_chunks = s_total // ch
        for ci in range(n_chunks):
            c0 = ci * ch
            t_pred = pred_pool.tile([128, ch * T], f32)
            nc.sync.dma_start(
                t_pred[0:BL, :],
                pred_d[:, c0:c0 + ch, :].rearrange("b s t -> b (s t)"))
            t_em = em_pool.tile([128, ch * 8], f32)
            for iq in range(4):
                em_src = bass.AP(tensor=pred_d, offset=c0 * T + iq * 8,
                                 ap=[[s_total * T, BL], [T, ch], [1, 8]])
                nc.sync.dma_start(t_em[iq * 32:(iq + 1) * 32, :], em_src)
            t_m = m_pool.tile([128, ch * 8], f32)
            if ci == 0:
                nc.sync.dma_start(t_m[:, 0:8], start128_d[:])

            for sl in range(ch):
                s = c0 + sl
                sm = s % 4
                # ---- pem transpose (DVE 32x32 blocks) + exp, per 4 steps
                if sm == 0:
                    t_pt = pem_pool.tile([32, 128], f32)
                    nc.vector.transpose(t_pt[:], t_pred[0:32, sl * T: sl * T + 128])
                    pem = pem_pool.tile([32, 128], f32)
                    nc.scalar.activation(pem[:], t_pt[:],
                                         mybir.ActivationFunctionType.Exp)
                pslice = pem[:, sm * 32:(sm + 1) * 32]

                # ---- denominator ----
                if s == 0:
                    nc.vector.tensor_mul(t_alpha[0:32, :], t_estart[:], pslice)
                else:
                    dp = dps.tile([128, 32], f32)
                    nc.tensor.matmul(dp[:], t_elhs[:], t_alpha[0:32, :],
                                     start=True, stop=True)
                    nc.vector.tensor_mul(t_alpha[0:32, :], dp[0:32, :], pslice)
                    if s % RENORM == 0:
                        zb = zps.tile([128, 32], f32)
                        nc.tensor.matmul(zb[:], t_ones[:], t_alpha[:],
                                         start=True, stop=True)
                        t_z = small_pool.tile([1, 32], f32)
                        nc.vector.tensor_copy(t_z[:], zb[0:1, :])
                        k = s // RENORM - 1
                        nc.sync.dma_start(z_d[k:k + 1, :], t_z[:])
                        t_rz = small_pool.tile([32, 32], f32)
                        nc.vector.reciprocal(t_rz[:], zb[0:32, :])
                        nc.vector.tensor_mul(t_alpha[0:32, :],
                                             t_alpha[0:32, :], t_rz[:])

                # ---- viterbi ----
                if s >= 1:
                    if sl >= 1:
                        m_prev = t_m[:, (sl - 1) * 8: sl * 8]
                        em_prev = t_em[:, (sl - 1) * 8: sl * 8]
                    else:
                        m_prev = prev_m[:, (ch - 1) * 8: ch * 8]
                        em_prev = prev_em[:, (ch - 1) * 8: ch * 8]
                    t_sc = sc_pool.tile([128, 8], f32)
                    nc.vector.tensor_add(t_sc[:], m_prev, em_prev)

                    cand = cand_ps.tile([128, 256], f32)
                    cp_inst = nc.scalar.copy(cand[:], t_trp[:])
                    sc_b = bass.AP(tensor=t_sc.tensor, offset=t_sc.offset,
                                   ap=[list(t_sc.ap[0]), [0, 8], [1, 8]])
                    for q in range(4):
                        out_q = bass.AP(tensor=cand.tensor,
                                        offset=cand.offset + q * 8,
                                        ap=[list(cand.ap[0]), [32, 8], [1, 8]])
                        mm = nc.tensor.matmul(out_q,
                                              t_dq4[:, q * 128:(q + 1) * 128],
                                              sc_b, start=False, stop=(q == 3),
                                              skip_group_check=True)
                        # start=False matmul reads PSUM: Tile does not model
                        # that read, so order it after the ScalarE trans-copy
                        tile.add_dep_helper(mm.ins, cp_inst.ins,
                                            reason="accumulate reads trans")
                    cand3 = bass.AP(tensor=cand.tensor, offset=cand.offset,
                                    ap=[list(cand.ap[0]), [32, 8], [1, 32]])
                    nc.vector.tensor_reduce(
                        t_m[:, sl * 8:(sl + 1) * 8], cand3,
                        axis=mybir.AxisListType.X, op=mybir.AluOpType.max)

            nc.sync.dma_start(mhist_d[:, c0 * 8:(c0 + ch) * 8], t_m[:])
            prev_m, prev_em = t_m, t_em

        nc.sync.dma_start(alpha_d[:], t_alpha[0:32, :])

    _split_sync_waits(nc)
    return nc


def _get_program(s_total=S, ch=CH):
    key = (s_total, ch)
    if key not in _cache:
        _apply_tile_patches()
        _cache[key] = _build_bass(s_total, ch)
    return _cache[key]


# ----------------------------------------------------------------------------
# host-side finish
# ----------------------------------------------------------------------------
def _host_finish(pred, labels, start, end, trans, m_all, alpha_all, z_all,
                 s_total=S):
    """m_all [NCORES, 128, s_total, 8]; alpha_all [NCORES, 32, 32] (j,b);
    z_all [NCORES, n_renorm, 32]."""
    f32 = np.float32
    bnum = pred.shape[0]
    # scores[s, bglobal, j] = m + em  (bit-exact fl add, same as device)
    m_r = m_all.reshape(NCORES, 4, 32, s_total, 8)          # [c, jq, b, s, jl]
    m_r = m_r.transpose(3, 0, 2, 1, 4).reshape(s_total, bnum, T)  # [s, B, j]
    scores = m_r + pred.transpose(1, 0, 2)                   # f32 add
    # backtrace
    bidx = np.arange(bnum)
    tags = np.zeros((bnum, s_total), np.int32)
    cur = (scores[s_total - 1] + end[None, :]).argmax(axis=1).astype(np.int32)
    tags[:, s_total - 1] = cur
    transT = np.ascontiguousarray(trans)                     # [i, j]
    for s in range(s_total - 2, -1, -1):
        # bp[s+1][b, cur] = argmax_i( scores[s][b, i] + trans[i, cur] )
        cand = scores[s] + transT[:, cur].T                  # [B, i]
        cur = cand.argmax(axis=1).astype(np.int32)
        tags[:, s] = cur

    # numerator (float64 accumulation; loss tolerance is loose)
    sc = start[labels[:, 0]].astype(np.float64) + \
        pred[bidx, 0, labels[:, 0]].astype(np.float64)
    trans_sc = trans[labels[:, :-1], labels[:, 1:]].astype(np.float64)
    em_sc = np.take_along_axis(pred[:, 1:], labels[:, 1:, None],
                               axis=2)[..., 0].astype(np.float64)
    num = sc + (trans_sc + em_sc).sum(axis=1) + \
        end[labels[:, s_total - 1]].astype(np.float64)

    # denominator finish: den[b] = c[b] + log(sum_j alpha[j,b] * exp(end[j]))
    den = np.zeros(bnum, np.float64)
    for c in range(NCORES):
        a = alpha_all[c].astype(np.float64)                  # [j, b]
        w = np.exp(end.astype(np.float64))[:, None]
        logc = np.log(z_all[c].astype(np.float64)).sum(axis=0)
        den[c * BL:(c + 1) * BL] = logc + np.log((a * w).sum(axis=0))

    loss = -np.mean(num - den)
    return tags, f32(loss)


# ----------------------------------------------------------------------------
# entry point
# ----------------------------------------------------------------------------
def kernel(pred, attention_mask, labels, start_transitions, end_transitions,
           transitions):
    pred = np.asarray(pred, np.float32)
    mask = np.asarray(attention_mask)
    labels = np.asarray(labels)
    start = np.asarray(start_transitions, np.float32)
    end = np.asarray(end_transitions, np.float32)
    trans = np.asarray(transitions, np.float32)

    if (pred.shape != (B, S, T) or mask.shape != (B, S)
            or labels.shape != (B, S) or not np.all(mask == 1)):
        return _np_reference(pred, mask, labels, start, end, trans)

    from concourse.bass_utils import run_bass_kernel_spmd

    nc = _get_program()
    statics = _build_statics(start, trans)
    in_maps = []
    for c in range(NCORES):
        im = {"pred": np.ascontiguousarray(pred[c * BL:(c + 1) * BL])}
        im.update(statics)
        in_maps.append(im)
    res = run_bass_kernel_spmd(nc, in_maps, core_ids=list(range(NCORES)))

    m_all = np.stack([r["mhist"].reshape(128, S, 8) for r in res.results])
    alpha_all = np.stack([r["alpha_out"] for r in res.results])
    z_all = np.stack([r["z_out"] for r in res.results])

    tags, loss = _host_finish(pred, labels, start, end, trans,
                              m_all, alpha_all, z_all)
    return tags, loss
